# revision 24
# baseline (speedup 1.0000x reference)
"""Multi-head self-attention (dense transformer block) on 8 TRN2 NeuronCores.

Data-parallel over batch: 8 batch items -> 8 cores, one image each, zero
collectives.  fp8(e4m3) DoubleRow matmuls halve the instruction count of
every contraction-bound phase (QKV projections contract C=512, PV contracts
S=1024, output projection contracts nh*dv=512): one DoubleRow instruction
contracts 256 rows (two 128-partition planes paired along a dim-1 axis of
both operands) in the same ~512-cycle stream as one bf16 matmul.  Scores
(K=64 per head) stay bf16 with the zero-padded-to-128 layout -- DoubleRow
gives no win there (out-column rate limited) and 64-partition fp8 measures
slower.

Scale plan (all power-of-2 so they cancel exactly):
  x8 = x (e4m3), w{q,k,v,o}8 = 16*W (e4m3, good dynamic range)
  qt = (Wq8^T x8) * 2^-4  -> exact-scale bf16 q        [heads on partitions]
  kt = (Wk8^T x8) * 2^-4  -> bf16 k, zero-padded per head
  v8 = x8^T Wv8 copied raw (= 16*v) into a templated fp8 tile
       [P kpos, 2 (kpos-chunk pair), NH, 128] where slot 0 of the last dim
       is a ones column (softmax denominator lands at pv partition 0,
       required by the base-partition-0 custom-DVE reciprocal), slots 1:64
       zero, slots 64:128 hold 16*v -> attnT lands at pv partitions 64:128
  est8 = exp(s/8)/4 in e4m3 ([0.001..61] well inside range), written either
       by ScalarE activation (scale=1/8, bias=-2ln2) or by a DVE Schraudolph
       (uint8 bits = s*1.4427 + 40, saturating at 0 kills the NaN-encoding
       wraparound for deeply negative scores), consistent encodings
  at8 = pv * (1/denom) = 16*attn (e4m3), heads at partitions 64:128 of four
       pair tiles, partitions 0:64 zeroed; Wo8 is host-permuted to match
  out = (Wo8^T at8) * 2^-8 + x  (exact fp32 residual)

Schedule: all 48 projection DoubleRow matmuls run as a dense prologue
(2-bank projection pool, closed before the pv pool opens).  The attention
loop is paced by the exp chain (ScalarE ~1.1us per [128,1024] step); scores
for step g+3 are pre-issued into a triple-buffered pool so exp never waits
on a PSUM WAR.  DVE Schraudolph takes ki 3/6 (and some ki 4) of the middle
heads -- ki >= 3 keeps those exps from queueing behind the previous head's
normalize (reciprocal + releasing multiply) in the DVE FIFO, and heads 0/1
and 7 stay DVE-clean so the prologue copy-outs and the tail normalize are
never queued behind Schraudolph work.  PV accumulation runs at ki 5/6/7.
The output projection opens six [128,512] psums over the freed scores
banks and accumulates pair-tiles 0..2 while head 7 normalizes.
"""

import math

import numpy as np

B = 8
C = 512
S = 1024
NH = 8
D = 64
P = 128
KO = C // P  # 4 partition tiles over the channel/contract dim
SO = S // P  # 8 partition tiles over positions
NQ = S // 512  # 2 free-dim chunks of 512 per matmul (PSUM bank limit)

_GRAPH_CACHE = {}

# attention steps whose exp runs on DVE (Schraudolph) instead of ScalarE
DVE_EXP = (
    {(h, ki) for h in range(2, 7) for ki in (4, 6)}
    | {(h, 5) for h in range(3, 7)}
    | {(7, 3), (7, 4)}  # h7: shortens the ScalarE chain's final head,
    # which gates the output projection; ki>=3 clears head 6's normalize
)


def _build_graph(with_bias: bool):
    import concourse.bass as bass
    import concourse.tile as tile
    from concourse import bacc, mybir
    from contextlib import ExitStack

    F32 = mybir.dt.float32
    BF16 = mybir.dt.bfloat16
    F8 = mybir.dt.float8e4
    U8 = mybir.dt.uint8
    Exp = mybir.ActivationFunctionType.Exp
    ADD = mybir.AluOpType.add
    MUL = mybir.AluOpType.mult
    DR = mybir.MatmulPerfMode.DoubleRow
    SCH_A = float(8.0 * math.log2(math.e) / 8.0)  # 1.442695
    SCH_B = 40.0
    EXP_BIAS = -2.0 * math.log(2.0)

    nc = bacc.Bacc("TRN2", target_bir_lowering=False, debug=False, num_devices=B)

    x = nc.declare_dram_parameter("x", [C, S], F32, isOutput=False)
    x8 = nc.declare_dram_parameter("x8", [P, KO, S], F8, isOutput=False)
    wq8 = nc.declare_dram_parameter("wq8", [P, KO, NH * D], F8, isOutput=False)
    wk8 = nc.declare_dram_parameter("wk8", [P, KO, NH * D], F8, isOutput=False)
    wv8 = nc.declare_dram_parameter("wv8", [P, KO, NH * D], F8, isOutput=False)
    wo8 = nc.declare_dram_parameter("wo8", [P, 2 * KO, C], F8, isOutput=False)
    zb16 = nc.declare_dram_parameter("zb16", [D, S], BF16, isOutput=False)
    if with_bias:
        bq = nc.declare_dram_parameter("bq", [NH * D], F32, isOutput=False)
        bk = nc.declare_dram_parameter("bk", [NH * D], F32, isOutput=False)
        bv16 = nc.declare_dram_parameter("bv16", [NH * D], F32, isOutput=False)
        bo = nc.declare_dram_parameter("bo", [C], F32, isOutput=False)
    out = nc.declare_dram_parameter("out", [C, S], F32, isOutput=True)

    x_r = x.rearrange("(ko p) s -> p ko s", p=P)

    with ExitStack() as ctx:
        tc = ctx.enter_context(tile.TileContext(nc))
        singles = ctx.enter_context(tc.tile_pool(name="singles", bufs=1))
        est_po = ctx.enter_context(tc.tile_pool(name="est_po", bufs=6))
        out_po = ctx.enter_context(tc.tile_pool(name="out_po", bufs=3))
        rr_po = ctx.enter_context(tc.tile_pool(name="rr_po", bufs=2))

        xb = singles.tile([P, KO, S], F32, tag="xb", name="xb")  # fp32 residual
        x8_sb = singles.tile([P, KO, S], F8, tag="x8", name="x8")
        wq_sb = singles.tile([P, KO, NH * D], F8, tag="wq", name="wq")
        wk_sb = singles.tile([P, KO, NH * D], F8, tag="wk", name="wk")
        wv_sb = singles.tile([P, KO, NH * D], F8, tag="wv", name="wv")
        wo_sb = singles.tile([P, 2 * KO, C], F8, tag="wo", name="wo")
        qt_sb = [singles.tile([P, S], BF16, tag=f"qt{m}", name=f"qt{m}") for m in range(KO)]
        kt_sb = [singles.tile([P, S], BF16, tag=f"kt{h}", name=f"kt{h}") for h in range(NH)]
        v8_sb = [
            singles.tile([P, 2, NH, 2 * D], F8, tag=f"v8_{p}", name=f"v8_{p}")
            for p in range(SO // 2)
        ]
        at8_sb = [
            singles.tile([P, 2, S], F8, tag=f"at{t}", name=f"at{t}") for t in range(KO)
        ]

        # ---- loads.  Keep the startup HBM footprint small so x8/Wq land
        # fast: V template built by GpSimd memsets, the 2MB fp32 residual
        # and at8 zero-halves deferred into the attention loop.
        _q = [nc.scalar, nc.sync]
        nc.scalar.dma_start(out=x8_sb[:], in_=x8[:])
        nc.scalar.dma_start(out=wq_sb[:], in_=wq8[:])
        nc.scalar.dma_start(out=wk_sb[:], in_=wk8[:])
        for hh in range(4):  # kt zero-halves for heads 0-3 first
            lo = 0 if hh % 2 else D
            nc.sync.dma_start(out=kt_sb[hh][lo : lo + D, :], in_=zb16[:])
        nc.sync.dma_start(out=wv_sb[:], in_=wv8[:])
        for hh in range(4, NH):
            lo = 0 if hh % 2 else D
            nc.sync.dma_start(out=kt_sb[hh][lo : lo + D, :], in_=zb16[:])
        nc.sync.dma_start(out=wo_sb[:], in_=wo8[:])
        for p in range(SO // 2):  # V template: ones column + zero block
            nc.gpsimd.memset(v8_sb[p][:, :, :, 0:1], 1.0)
            nc.gpsimd.memset(v8_sb[p][:, :, :, 1:D], 0.0)

        ebias = singles.tile([P, 1], F32, tag="ebias")
        nc.vector.memset(ebias[:], EXP_BIAS)

        if with_bias:
            bq_sb = singles.tile([P, KO, 1], F32, tag="bq")
            bk_sb = singles.tile([P, KO, 1], F32, tag="bk")
            nc.sync.dma_start(out=bq_sb[:, :, 0], in_=bq.rearrange("(ko p) -> p ko", p=P))
            nc.sync.dma_start(out=bk_sb[:, :, 0], in_=bk.rearrange("(ko p) -> p ko", p=P))
            bv_rep = singles.tile([P, NH * D], F32, tag="bv")
            _bv_ap = bv16.ap()
            nc.sync.dma_start(
                out=bv_rep[:],
                in_=bass.AP(
                    tensor=_bv_ap.tensor, offset=_bv_ap.offset, ap=[[0, P], [1, NH * D]]
                ),
            )
            bo_sb = singles.tile([P, KO, 1], F32, tag="bo")
            nc.sync.dma_start(out=bo_sb[:, :, 0], in_=bo.rearrange("(ko p) -> p ko", p=P))

        # PSUM: scores pool [128,1024] bufs=2 (4 banks) + 2-bank projection
        # pool kept open through the attention loop (projection groups weave
        # into the steps, so scores never queue behind a prologue wall) +
        # 2-bank pv pool.
        st_ctx = tc.tile_pool(name="st_ps", bufs=2, space="PSUM")
        pj_ctx = tc.tile_pool(name="pj_ps", bufs=2, space="PSUM")
        st_ps = st_ctx.__enter__()
        pj_ps = pj_ctx.__enter__()

        def proj_mms(ps, w_sb, mo, qc):
            """One QT/KT projection psum group: 2 fp8 DoubleRow matmuls."""
            for j in range(2):
                nc.tensor.matmul(
                    ps[:],
                    w_sb[:, 2 * j : 2 * j + 2, mo * P : (mo + 1) * P],
                    x8_sb[:, 2 * j : 2 * j + 2, qc * 512 : (qc + 1) * 512],
                    start=(j == 0),
                    stop=(j == 1),
                    perf_mode=DR,
                )

        def q_proj(mo, qc):
            ps = pj_ps.tile([P, 512], F32, tag="pjps", name=f"pjq{mo}_{qc}")
            proj_mms(ps, wq_sb, mo, qc)
            dst = qt_sb[mo][:, qc * 512 : (qc + 1) * 512]
            if with_bias:
                nc.vector.tensor_scalar(
                    out=dst, in0=ps[:], scalar1=1.0 / 16.0, scalar2=bq_sb[:, mo],
                    op0=MUL, op1=ADD,
                )
            else:
                nc.vector.tensor_scalar_mul(out=dst, in0=ps[:], scalar1=1.0 / 16.0)

        def k_proj(mo, qc):
            ps = pj_ps.tile([P, 512], F32, tag="pjps", name=f"pjk{mo}_{qc}")
            proj_mms(ps, wk_sb, mo, qc)
            # head 2mo data at rows 0:64 of kt[2mo]; head 2mo+1 at rows 64:128
            for half in range(2):
                hh = 2 * mo + half
                hrr = half * D
                dsth = kt_sb[hh][hrr : hrr + D, qc * 512 : (qc + 1) * 512]
                if with_bias:
                    nc.vector.tensor_scalar(
                        out=dsth, in0=ps[hrr : hrr + D], scalar1=1.0 / 16.0,
                        scalar2=bk_sb[hrr : hrr + D, mo], op0=MUL, op1=ADD,
                    )
                else:
                    nc.vector.tensor_scalar_mul(
                        out=dsth, in0=ps[hrr : hrr + D], scalar1=1.0 / 16.0
                    )

        def v_proj(so):
            ps = pj_ps.tile([P, 512], F32, tag="pjps", name=f"pjv{so}")
            for j in range(2):
                nc.tensor.matmul(
                    ps[:],
                    x8_sb[:, 2 * j : 2 * j + 2, so * P : (so + 1) * P],
                    wv_sb[:, 2 * j : 2 * j + 2, :],
                    start=(j == 0),
                    stop=(j == 1),
                    perf_mode=DR,
                )
            # ps = 16*v as (h, dv); slot dv+64 of the templated v8 tile
            dst = v8_sb[so // 2][:, so % 2, :, D : 2 * D]
            src = ps[:].rearrange("p (h d) -> p h d", h=NH)
            if with_bias:
                nc.vector.tensor_tensor(
                    dst, src, bv_rep[:].rearrange("p (h d) -> p h d", h=NH), ADD
                )
            else:
                nc.vector.tensor_copy(out=dst, in_=src)

        def st_mms(h, ki):
            st = st_ps.tile([P, S], F32, tag="stps", name=f"st{h}_{ki}")
            for qc in range(NQ):
                nc.tensor.matmul(
                    st[:, qc * 512 : (qc + 1) * 512],
                    kt_sb[h][:, ki * P : (ki + 1) * P],
                    qt_sb[h // 2][:, qc * 512 : (qc + 1) * 512],
                    start=True,
                    stop=True,
                )
            return st

        def normalize(h, pv):
            # pv row 0 = softmax denominator, rows 64:128 = 16*attnT; the
            # DVE multiply is the last pv reader and releases the bank
            t, j = h // 2, h % 2
            rrow = rr_po.tile([1, S], F32, tag="rrow")
            nc.vector.reciprocal_approx_fast(out=rrow[:], in_=pv[0:1, :])
            rrep = rr_po.tile([D, S], F32, tag="rrep")
            nc.gpsimd.partition_broadcast(rrep[:], rrow[0:1, :])
            nc.vector.tensor_tensor(
                at8_sb[t][D:P, j, :], pv[D:P, :], rrep[:], MUL
            )

        # ---- prologue: head-0 data first, then the rest of the projections
        # run dense on the PE while the exp chain starts.
        for qc in range(NQ):
            q_proj(0, qc)
        for qc in range(NQ):
            k_proj(0, qc)
        sts = {0: st_mms(0, 0), 1: st_mms(0, 1)}
        pv_ctx = tc.tile_pool(name="pv_ps", bufs=1, space="PSUM")
        pv_ps = pv_ctx.__enter__()

        # V and the mo1-3 projections weave into the loop (one group per
        # step), so the PE FIFO always alternates projection and scores work
        filler = {}
        for so in range(SO):
            filler.setdefault(so, []).append(lambda s=so: v_proj(s))
        for i, (fn, mo) in enumerate(
            [(k_proj, 1), (q_proj, 1), (k_proj, 2), (q_proj, 2), (k_proj, 3), (q_proj, 3)]
        ):
            base = (8, 10, 16, 18, 24, 26)[i]
            for qc in range(NQ):
                filler.setdefault(base + qc, []).append(
                    lambda f=fn, m=mo, q=qc: f(m, q)
                )
        for k in range(KO):  # fp32 residual (2MB) mid-attention
            filler.setdefault(24 + 2 * k, []).append(
                lambda kk=k: nc.sync.dma_start(out=xb[:, kk, :], in_=x_r[:, kk])
            )
        for t in range(KO):  # at8 zero-halves (needed at the tail)
            filler.setdefault(34 + 2 * t, []).append(
                lambda tt=t: nc.gpsimd.memset(at8_sb[tt][0:D, :, :], 0.0)
            )

        # ---- software-pipelined attention.  PV DoubleRow accumulation is
        # deferred to ki 5/6/7 (sum over kpos pairs commutes) so the single
        # pv bank is first written ~6us after the previous head's releasing
        # multiply -- no WAR stall.
        GT = NH * SO
        pv_cur = None
        est_h = {}
        for g in range(GT):
            h, ki = divmod(g, SO)
            if ki == 0:
                pv_cur = pv_ps.tile([P, S], F32, tag="pvps", name=f"pv{h}")
                est_h.clear()
            if ki % 2 == 0:
                est_h[ki // 2] = est_po.tile(
                    [P, 2, S], F8, tag="est", name=f"est{h}_{ki}"
                )
            eslot = est_h[ki // 2][:, ki % 2, :]
            if (h, ki) in DVE_EXP:
                nc.vector.tensor_scalar(
                    out=eslot.bitcast(U8),
                    in0=sts.pop(g)[:],
                    scalar1=SCH_A,
                    scalar2=SCH_B,
                    op0=MUL,
                    op1=ADD,
                )
            else:
                nc.scalar.activation(
                    out=eslot,
                    in_=sts.pop(g)[:],
                    func=Exp,
                    scale=1.0 / 8.0,
                    bias=ebias[:, 0:1],
                )
            if g + 2 < GT:
                h2, k2 = divmod(g + 2, SO)
                sts[g + 2] = st_mms(h2, k2)
            for fn in filler.get(g, ()):
                fn()
            for p in {5: (0, 1), 6: (2,), 7: (3,)}.get(ki, ()):
                for qc in range(NQ):
                    nc.tensor.matmul(
                        pv_cur[:, qc * 512 : (qc + 1) * 512],
                        v8_sb[p][:, :, h, :],
                        est_h[p][:, :, qc * 512 : (qc + 1) * 512],
                        start=(p == 0),
                        stop=(p == SO // 2 - 1),
                        perf_mode=DR,
                    )
            if ki == SO - 1:
                normalize(h, pv_cur)

        # ---- output projection + residual.  Six [128,512] psums open over
        # the freed scores banks; pair-tiles 0..2 accumulate while head 7
        # normalizes, then each chunk closes with its t=3 matmul + add + DMA.
        pv_ctx.__exit__(None, None, None)
        pj_ctx.__exit__(None, None, None)
        st_ctx.__exit__(None, None, None)
        po_ctx = tc.tile_pool(name="po_ps", bufs=6, space="PSUM")
        po_ps = po_ctx.__enter__()
        out_r = out.rearrange("(mo p) s -> p mo s", p=P)

        def po_mm(ps, mo, qc, t):
            nc.tensor.matmul(
                ps[:],
                wo_sb[:, 2 * t : 2 * t + 2, mo * P : (mo + 1) * P],
                at8_sb[t][:, :, qc * 512 : (qc + 1) * 512],
                start=(t == 0),
                stop=(t == KO - 1),
                perf_mode=DR,
            )

        def po_close(ps, mo, qc):
            ot = out_po.tile([P, 512], F32, tag="ot")
            # ot = psum * 2^-8 + x + bo
            if with_bias:
                nc.vector.tensor_scalar(
                    out=ot[:], in0=ps[:], scalar1=1.0 / 256.0, scalar2=bo_sb[:, mo],
                    op0=MUL, op1=ADD,
                )
                nc.vector.tensor_add(
                    out=ot[:], in0=ot[:],
                    in1=xb[:, mo, qc * 512 : (qc + 1) * 512],
                )
            else:
                nc.vector.scalar_tensor_tensor(
                    out=ot[:],
                    in0=ps[:],
                    scalar=1.0 / 256.0,
                    in1=xb[:, mo, qc * 512 : (qc + 1) * 512],
                    op0=MUL,
                    op1=ADD,
                )
            _q[(mo * NQ + qc) % 2].dma_start(
                out=out_r[:, mo, qc * 512 : (qc + 1) * 512], in_=ot[:]
            )

        chunks = [(mo, qc) for mo in range(KO) for qc in range(NQ)]
        po_tiles = {}
        for mo, qc in chunks[:6]:
            ps = po_ps.tile([P, 512], F32, tag="pops", name=f"po{mo}_{qc}")
            po_tiles[(mo, qc)] = ps
            for t in range(KO - 1):
                po_mm(ps, mo, qc, t)
        for mo, qc in chunks[:6]:
            ps = po_tiles[(mo, qc)]
            po_mm(ps, mo, qc, KO - 1)
            po_close(ps, mo, qc)
        for mo, qc in chunks[6:]:
            ps = po_ps.tile([P, 512], F32, tag="pops", name=f"po{mo}_{qc}")
            for t in range(KO):
                po_mm(ps, mo, qc, t)
            po_close(ps, mo, qc)
        po_ctx.__exit__(None, None, None)

    nc.compile()
    return nc


def _get_graph(with_bias: bool):
    key = bool(with_bias)
    if key not in _GRAPH_CACHE:
        _GRAPH_CACHE[key] = _build_graph(key)
    return _GRAPH_CACHE[key]


def _make_in_maps(inputs, with_bias: bool):
    import ml_dtypes

    e4 = np.dtype(ml_dtypes.float8_e4m3fn)
    f32 = np.float32

    def to8(a):
        return np.ascontiguousarray(np.clip(a, -240.0, 240.0).astype(e4))

    x = np.ascontiguousarray(np.asarray(inputs["x"], dtype=f32))
    assert x.shape == (B, C, 32, 32), x.shape
    xf = x.reshape(B, C, S)
    # x8[p, ko, s] = x[ko*128+p, s]
    x8 = xf.reshape(B, KO, P, S).transpose(0, 2, 1, 3)

    def wre(w):  # [C, N] -> [P, KO, N] with c = ko*128+p, scaled by 16
        a = np.asarray(w, dtype=f32) * 16.0
        return to8(a.reshape(KO, P, -1).transpose(1, 0, 2))

    ws = {
        "wq8": wre(inputs["Wq"]),
        "wk8": wre(inputs["Wk"]),
        "wv8": wre(inputs["Wv"]),
    }
    # wo8[p, s, c] = 16*Wo[s*64 + (p-64), c] for p >= 64 else 0
    wo = np.asarray(inputs["Wo"], dtype=f32) * 16.0  # [NH*D, C]
    wo8 = np.zeros((P, 2 * KO, C), dtype=f32)
    wo8[D:P, :, :] = wo.reshape(2 * KO, D, C).transpose(1, 0, 2)
    ws["wo8"] = to8(wo8)
    ws["zb16"] = np.zeros((D, S), dtype=ml_dtypes.bfloat16)

    maps = []
    for b in range(B):
        m = {
            "x": np.ascontiguousarray(xf[b]),
            "x8": to8(x8[b]),
        }
        m.update(ws)
        if with_bias:
            m["bq"] = np.ascontiguousarray(np.asarray(inputs["bq"], dtype=f32))
            m["bk"] = np.ascontiguousarray(np.asarray(inputs["bk"], dtype=f32))
            m["bv16"] = np.ascontiguousarray(np.asarray(inputs["bv"], dtype=f32) * 16.0)
            m["bo"] = np.ascontiguousarray(np.asarray(inputs["bo"], dtype=f32))
        maps.append(m)
    return maps


def _run(inputs, **spmd_kwargs):
    from concourse.bass_utils import run_bass_kernel_spmd

    nh = int(np.asarray(inputs.get("num_heads", NH)))
    assert nh == NH, f"kernel hardcodes num_heads={NH}, got {nh}"
    with_bias = any(
        np.any(np.asarray(inputs[k])) for k in ("bq", "bk", "bv", "bo") if k in inputs
    )
    nc = _get_graph(with_bias)
    in_maps = _make_in_maps(inputs, with_bias)
    res = run_bass_kernel_spmd(nc, in_maps, core_ids=list(range(B)), **spmd_kwargs)
    outs = np.stack([res.results[b]["out"] for b in range(B)])  # [B, C, S]
    return outs.reshape(B, C, 32, 32).astype(np.float32), res


def kernel(**inputs):
    out, _ = _run(inputs)
    return out
